# revision 9
# baseline (speedup 1.0000x reference)
"""DotAttention (soft-window biased) TRN2 Bass kernel.

Data-parallel over batch: B=32 split as 4 batches/core across 8 NeuronCores.
Per core: S = Q K^T (fp32r matmuls), logit = s*S + j/32 (the row-constant
-i/32 part of the reference's soft-window bias cancels in softmax),
P = exp(logit - SHIFT) with SHIFT=(T-1)/32 standing in for the row max
(exp args stay in fp32 range and underflow behavior matches the reference),
scores = P/rowsum(P) via ACT accum_out + reciprocal, summaries = scores @ V
via PE-transposed unnormalized score tiles with the 1/rowsum folded into the
PSUM evacuation (per-partition ACT scale).

`kernel(**inputs)` takes the full unsharded inputs and returns
(summaries, scores) like the reference.
"""

from contextlib import ExitStack

import numpy as np

B, T, D, DV = 32, 2048, 512, 512
M = 8
B_SH = B // M
SCALE = 1.0 / float(np.sqrt(D))

_CACHE = {}


def build(B_sh=B_SH, T_=T, D_=D, DV_=DV, scale=SCALE):
    import concourse.tile as tile
    from concourse import bacc, mybir

    f32 = mybir.dt.float32
    f32r = mybir.dt.float32r

    NT = T_ // 128    # seq tiles (i and j)
    ND = D_ // 128    # contraction tiles for QK^T
    JC = min(512, T_)  # j-chunk = one PSUM bank of fp32
    NJC = T_ // JC
    NTC = JC // 128   # 128-tiles per j-chunk
    shift = (T_ - 1) / 32.0
    assert T_ % 128 == 0 and D_ % 128 == 0 and ND <= 4 and DV_ <= 512

    nc = bacc.Bacc("TRN2", target_bir_lowering=False, debug=False)
    Q = nc.dram_tensor("queries", [B_sh, T_, D_], f32, kind="ExternalInput").ap()
    K = nc.dram_tensor("keys", [B_sh, T_, D_], f32, kind="ExternalInput").ap()
    V = nc.dram_tensor("values", [B_sh, T_, DV_], f32, kind="ExternalInput").ap()
    WB = nc.dram_tensor("wbias", [1, T_], f32, kind="ExternalInput").ap()
    ID = nc.dram_tensor("ident", [128, 128], f32, kind="ExternalInput").ap()
    SC = nc.dram_tensor("scores", [B_sh, T_, T_], f32, kind="ExternalOutput").ap()
    SM = nc.dram_tensor("summaries", [B_sh, T_, DV_], f32, kind="ExternalOutput").ap()

    with tile.TileContext(nc) as tc, ExitStack() as ctx:
        singles = ctx.enter_context(tc.tile_pool(name="singles", bufs=1))
        kv_pool = ctx.enter_context(tc.tile_pool(name="kv", bufs=2))
        kstage = ctx.enter_context(tc.tile_pool(name="kstage", bufs=3))
        qpool = ctx.enter_context(tc.tile_pool(name="qpool", bufs=2))
        ppool = ctx.enter_context(tc.tile_pool(name="ppool", bufs=2))
        ptpool = ctx.enter_context(tc.tile_pool(name="ptpool", bufs=2))
        opool = ctx.enter_context(tc.tile_pool(name="opool", bufs=2))
        stat = ctx.enter_context(tc.tile_pool(name="stat", bufs=4))
        ps_s = ctx.enter_context(tc.tile_pool(name="ps_s", bufs=3, space="PSUM"))
        ps_t = ctx.enter_context(tc.tile_pool(name="ps_t", bufs=3, space="PSUM"))
        ps_o = ctx.enter_context(tc.tile_pool(name="ps_o", bufs=2, space="PSUM"))

        ident = singles.tile([128, 128], f32r)
        nc.sync.dma_start(out=ident, in_=ID.bitcast(f32r))
        bias_t = singles.tile([128, 1], f32)
        nc.vector.memset(bias_t, -float(shift))
        wb_t = singles.tile([128, T_], f32)
        nc.gpsimd.dma_start(out=wb_t, in_=WB.to_broadcast((128, T_)))

        def kv_alloc():
            # resident K^T (f32r): kt[:, d, :] is the d-th [128, T] slab and
            # resident V (f32r): vt[:, j, :] is the j-th [128, DV] tile
            kt = kv_pool.tile([128, ND, T_], f32r, name="kt", tag="kt")
            vt = kv_pool.tile([128, NT, DV_], f32r, name="vt", tag="vt")
            return kt, vt

        def kv_stage_dma(vt, b, j):
            nc.sync.dma_start(
                out=vt[:, j, :], in_=V[b, 128 * j : 128 * (j + 1), :].bitcast(f32r)
            )
            kst = kstage.tile([128, D_], f32r, name="kst", tag="kst")
            nc.sync.dma_start(
                out=kst, in_=K[b, 128 * j : 128 * (j + 1), :].bitcast(f32r)
            )
            return kst

        def kv_stage_transpose(kt, kst, j):
            pstk = ps_t.tile([128, 4, 128], f32r, name="pstk", tag="ps_t")
            for d in range(ND):
                nc.tensor.transpose(
                    pstk[:, d, :], kst[:, 128 * d : 128 * (d + 1)], ident
                )
            nc.vector.tensor_copy(kt[:, :, 128 * j : 128 * (j + 1)], pstk[:, :ND, :])

        kt_next, vt_next = kv_alloc()
        for j in range(NT):
            kst0 = kv_stage_dma(vt_next, 0, j)
            kv_stage_transpose(kt_next, kst0, j)

        for b in range(B_sh):
            kt, vt = kt_next, vt_next
            if b + 1 < B_sh:
                kt_next, vt_next = kv_alloc()

            def q_load(it):
                i0 = 128 * it
                qst = qpool.tile([128, D_], f32r, name="qst", tag="qst")
                nc.sync.dma_start(out=qst, in_=Q[b, i0 : i0 + 128, :].bitcast(f32r))
                return qst

            def q_transpose(qst):
                pstq = ps_t.tile([128, 4, 128], f32r, name="pstq", tag="ps_t")
                for d in range(ND):
                    nc.tensor.transpose(
                        pstq[:, d, :], qst[:, 128 * d : 128 * (d + 1)], ident
                    )
                qt = qpool.tile([128, ND, 128], f32r, name="qt", tag="qt")
                nc.vector.tensor_copy(qt, pstq[:, :ND, :])
                return qt

            qst_next = q_load(0)
            qt = q_transpose(qst_next)
            for it in range(NT):
                i0 = 128 * it
                if it + 1 < NT:
                    qst_next = q_load(it + 1)
                if b + 1 < B_sh:
                    kst_next = kv_stage_dma(vt_next, b + 1, it)

                # S = Q K^T by j-chunks; +window bias in PSUM; exp to SBUF.
                # Transpose each chunk's unnormalized P tiles immediately so
                # PE never waits for the row-sum/normalize chain.
                p_un = ppool.tile([128, T_], f32r, name="p_un", tag="p_un")
                pt = ptpool.tile([128, NT, 128], f32r, name="pt", tag="pt")
                sums4 = stat.tile([128, NJC], f32, name="sums4", tag="sums4")
                for jc in range(NJC):
                    ps = ps_s.tile([128, JC], f32, name="ps", tag="ps_s")
                    for d in range(ND):
                        nc.tensor.matmul(
                            ps,
                            qt[:, d, :],
                            kt[:, d, JC * jc : JC * (jc + 1)],
                            start=(d == 0),
                            stop=(d == ND - 1),
                        )
                    nc.vector.tensor_tensor(
                        out=ps,
                        in0=ps,
                        in1=wb_t[:, JC * jc : JC * (jc + 1)],
                        op=mybir.AluOpType.add,
                    )
                    nc.scalar.activation(
                        out=p_un[:, JC * jc : JC * (jc + 1)],
                        in_=ps,
                        func=mybir.ActivationFunctionType.Exp,
                        bias=bias_t,
                        scale=float(scale),
                        accum_out=sums4[:, jc : jc + 1],
                    )
                    pstp = ps_t.tile([128, 4, 128], f32r, name="pstp", tag="ps_t")
                    for u in range(NTC):
                        j0 = JC * jc + 128 * u
                        nc.tensor.transpose(
                            pstp[:, u, :], p_un[:, j0 : j0 + 128], ident
                        )
                    nc.vector.tensor_copy(
                        pt[:, NTC * jc : NTC * (jc + 1), :], pstp[:, :NTC, :]
                    )
                    if jc == 0 and it + 1 < NT:
                        qt_next = q_transpose(qst_next)

                if b + 1 < B_sh:
                    kv_stage_transpose(kt_next, kst_next, it)

                sums = stat.tile([128, 1], f32, name="sums", tag="sums")
                nc.vector.reduce_sum(out=sums, in_=sums4, axis=mybir.AxisListType.X)
                recip = stat.tile([128, 1], f32, name="recip", tag="recip")
                nc.vector.reciprocal(recip, sums)

                # summaries: O = (P^T)^T V, scaled by 1/rowsum on evacuation
                pso = ps_o.tile([128, DV_], f32, name="pso", tag="ps_o")
                for jt in range(NT):
                    nc.tensor.matmul(
                        pso,
                        pt[:, jt, :],
                        vt[:, jt, :],
                        start=(jt == 0),
                        stop=(jt == NT - 1),
                    )
                o_t = opool.tile([128, DV_], f32, name="o_t", tag="o_t")
                nc.scalar.activation(
                    out=o_t,
                    in_=pso,
                    func=mybir.ActivationFunctionType.Copy,
                    bias=0.0,
                    scale=recip,
                )
                nc.sync.dma_start(out=SM[b, i0 : i0 + 128, :], in_=o_t)

                # scores: normalize in place on ACT in chunk pieces (short
                # ops so the next tile's EXPs aren't head-of-line blocked)
                for jc in range(NJC):
                    sl = slice(JC * jc, JC * (jc + 1))
                    nc.scalar.activation(
                        out=p_un[:, sl],
                        in_=p_un[:, sl],
                        func=mybir.ActivationFunctionType.Copy,
                        bias=0.0,
                        scale=recip,
                    )
                    nc.sync.dma_start(
                        out=SC[b, i0 : i0 + 128, sl], in_=p_un[:, sl].bitcast(f32)
                    )
                if it + 1 < NT:
                    qt = qt_next

    nc.compile()
    return nc


def make_wbias(T_=T, scale=SCALE):
    j = np.arange(T_, dtype=np.float64)
    return ((j / 32.0) / scale).astype(np.float32).reshape(1, T_)


def _get_nc():
    if "nc" not in _CACHE:
        _CACHE["nc"] = build()
    return _CACHE["nc"]


def run(queries, keys, values, trace=False):
    from concourse.bass_utils import run_bass_kernel_spmd

    nc = _get_nc()
    wb = make_wbias()
    ident_np = np.eye(128, dtype=np.float32)
    queries = np.ascontiguousarray(queries, dtype=np.float32)
    keys = np.ascontiguousarray(keys, dtype=np.float32)
    values = np.ascontiguousarray(values, dtype=np.float32)
    in_maps = [
        {
            "queries": queries[c * B_SH : (c + 1) * B_SH],
            "keys": keys[c * B_SH : (c + 1) * B_SH],
            "values": values[c * B_SH : (c + 1) * B_SH],
            "wbias": wb,
            "ident": ident_np,
        }
        for c in range(M)
    ]
    res = run_bass_kernel_spmd(nc, in_maps, core_ids=list(range(M)), trace=trace)
    summaries = np.concatenate([res.results[c]["summaries"] for c in range(M)], axis=0)
    scores = np.concatenate([res.results[c]["scores"] for c in range(M)], axis=0)
    return (summaries, scores), res


def kernel(queries, keys, values):
    (summaries, scores), _ = run(queries, keys, values, trace=False)
    return (summaries, scores)
